# revision 12
# baseline (speedup 1.0000x reference)
"""Trainium2 Bass kernel for nn_Autoencoder_78288663872074.

Autoencoder MLP: 32 Dense(ELU) layers, widths <= 5, over a 4194304x5 batch.

Strategy
--------
- Pure data parallel over 8 cores (524288 rows each).
- On each core, rows are packed into a block-diagonal layout: 25 independent
  5-feature blocks on 125 partitions x 512 batch columns per tile
  (12800 rows/tile).  Each Dense layer is then ONE 128x128 matmul against a
  block-diagonal weight matrix (25 copies of the 5x5 padded layer weight).
- ELU(u) = relu(u) + (min(exp(u), 1) - 1) is kept as TWO zero-centered
  tensors r and e1 whose sum is exactly the activation (both small -> safe
  to round to fp32r; never carry a +1 offset in fp32r).  Per layer:
     PE  : z = Wblk^T e1_prev + Wblk^T r_prev   (PSUM accumulate, fp32r)
     ACT : e_raw = Exp(z + B)                    (bias is free on ACT)
     DVE : r  = max(z + B, 0)                    (dual-op tensor_scalar)
     POOL: e1 = min(e_raw, 1) - 1                (gpsimd dual-op tensor_scalar)
  so the four engines each carry one pass per layer; K_ILV pairs of tiles are
  interleaved layer-by-layer so the engines always see independent work.
- Matmuls use float32r (1 PE cycle/row vs 4 for fp32; ~3e-4 per-store
  rounding -> ~1.6e-3 final norm rel err).
- Entry/exit to the block layout is done with PE transpose(-matmuls) against
  identity matrices, so all DRAM DMA is fully contiguous.
"""

import sys

sys.path.insert(0, "/opt/trn_rl_repo")

import numpy as np

import concourse.bass as bass
import concourse.mybir as mybir
import concourse.tile as tile
from concourse.vector_clock import ScopedClock

# ---------------------------------------------------------------- constants
BATCH = 4_194_304
D = 5
N_CORES = 8
ROWS_PER_CORE = BATCH // N_CORES          # 524288
NB = 25                                   # 5-feature blocks per matmul
P_USED = NB * D                           # 125 partitions carry data
NFREE = 512                               # batch columns per tile
TILE_ROWS = NB * NFREE                    # 12800 rows per tile
PAIR_ROWS = 2 * TILE_ROWS                 # 25600 rows per pair
G_FREE = 100 * D                          # 500 fp32 per partition per tile
N_LAYERS = 32

F32 = mybir.dt.float32
F32R = mybir.dt.float32r  # TF32-like: PE runs 1 cycle/row (vs 4 for fp32) at N>=256


# ------------------------------------------------------- tail-drain fix
def _patch_tile_drain():
    """walrus in this container accepts a single sync-wait per instruction;
    spread the kernel-tail drain waits over one-wait SP nops."""

    def _drain_and_barrier_split(self, tick_clock, wait_clock):
        nc = self.nc
        probe = nc.sync.nop(nofuse=True)
        wait_clock.add_sem_waits(
            probe.ins, ScopedClock({None: tick_clock.global_clock})
        )
        si = probe.ins.sync_info
        ow = list(si.on_wait or []) if si else []
        if len(ow) > 1:
            si.on_wait = ow[:1]
            for w in ow[1:]:
                n = nc.sync.nop(nofuse=True)
                nsi = n.ins.sync_info
                if nsi is None:
                    n.ins.sync_info = mybir.SyncInfo(on_wait=[w], on_update=[])
                else:
                    nsi.on_wait = [w]
        nc.sync.drain()
        nc.all_engine_barrier()
        assert self.sems is not None
        popped = nc._tile_sem_poison_stack.pop()
        assert popped is self._sem_poison
        nc.clear_and_free_semaphores(list(self.sems.allocated().values()))
        nc.all_engine_barrier()

    tile.TileContext._drain_and_barrier = _drain_and_barrier_split


_patch_tile_drain()


def _split_multiwaits(nc):
    """Hoist extra sync-waits onto same-engine NOPs (1 wait per instruction)."""
    n_split = 0
    for fn in nc.m.functions:
        for blk in fn.blocks:
            new = []
            for inst in blk.instructions:
                si = inst.sync_info
                ow = list(si.on_wait) if (si and si.on_wait) else []
                if len(ow) > 1:
                    for i, w in enumerate(ow[:-1]):
                        new.append(
                            mybir.InstNoOp(
                                name=f"{inst.name}-wsplit{i}",
                                engine=inst.engine,
                                ins=[],
                                outs=[],
                                sync_info=mybir.SyncInfo(on_wait=[w], on_update=[]),
                            )
                        )
                        n_split += 1
                    si.on_wait = [ow[-1]]
                new.append(inst)
            blk.instructions[:] = new
    return n_split


# ------------------------------------------------------- host-side weights
def _layer_list(enc_params, dec_params):
    layers = []
    for params in (enc_params, dec_params):
        for W, b in params:
            layers.append((np.asarray(W, np.float32), np.asarray(b, np.float32)))
    assert len(layers) == N_LAYERS
    return layers


def _host_constants(enc_params, dec_params):
    """Build block-diag weights, corrected per-partition biases, selectors."""
    layers = _layer_list(enc_params, dec_params)

    wblk = np.zeros((128, N_LAYERS * 128), np.float32)  # sbuf layout [p, l*128+m]
    bvec = np.zeros((128, N_LAYERS), np.float32)
    for l, (W, b) in enumerate(layers):
        d_in, d_out = W.shape
        W5 = np.zeros((D, D), np.float32)
        W5[:d_in, :d_out] = W
        b5 = np.zeros((D,), np.float32)
        b5[:d_out] = b
        B5 = b5  # activations are stored as (e1-1, r); their sum is exactly ELU
        blk = np.zeros((128, 128), np.float32)
        for bi in range(NB):
            blk[D * bi : D * bi + D, D * bi : D * bi + D] = W5
        wblk[:, l * 128 : (l + 1) * 128] = blk
        bvec[:P_USED, l] = np.tile(B5, NB)

    ident = np.zeros((128, 128), np.float32)
    np.fill_diagonal(ident[:128, :128], 1.0)

    invsel = ident.copy()
    return wblk, bvec, ident, invsel


# ------------------------------------------------------- device program
K_ILV = 3          # pairs interleaved layer-by-layer (PSUM-slot limited)
MIN_ON_DVE = 0.0   # fraction of layers whose min() runs on DVE instead of POOL


def _build_bass(rows_per_core):
    n_pairs = (rows_per_core + PAIR_ROWS - 1) // PAIR_ROWS
    assert rows_per_core >= PAIR_ROWS

    nc = bass.Bass(
        "TRN2",
        target_bir_lowering=False,
        debug=False,
        enable_asserts=False,
        num_devices=1,
    )
    xs = nc.dram_tensor("xs", [rows_per_core * D + 4], F32, kind="ExternalInput")
    wd = nc.dram_tensor("wblk", [128, N_LAYERS * 128], F32R, kind="ExternalInput")
    bd = nc.dram_tensor("bias", [128, N_LAYERS], F32, kind="ExternalInput")
    idd = nc.dram_tensor("ident", [128, 128], F32, kind="ExternalInput")
    ivd = nc.dram_tensor("invsel", [128, 128], F32, kind="ExternalInput")
    ys = nc.dram_tensor("ys", [rows_per_core * D], F32, kind="ExternalOutput")

    with tile.TileContext(nc) as tc:
        with (
            tc.tile_pool(name="const", bufs=1) as const,
            tc.tile_pool(name="g", bufs=4 * K_ILV) as gpool,
            tc.tile_pool(name="x0sb", bufs=K_ILV + 1) as x0pool,
            tc.tile_pool(name="eraw", bufs=2 * K_ILV + 1) as epool,
            tc.tile_pool(name="rr", bufs=2 * K_ILV + 2) as rpool,
            tc.tile_pool(name="e1", bufs=2 * K_ILV + 2) as e1pool,
            tc.tile_pool(name="hsb", bufs=K_ILV + 1) as hpool,
            tc.tile_pool(name="ps", bufs=3, space="PSUM") as ps,
            tc.tile_pool(name="pss", bufs=2, space="PSUM") as pss,
        ):
            wsb = const.tile([128, N_LAYERS * 128], F32R)
            nc.sync.dma_start(out=wsb[:], in_=wd[:])
            bsb = const.tile([128, N_LAYERS], F32)
            nc.sync.dma_start(out=bsb[:], in_=bd[:])
            isb = const.tile([128, 128], F32)
            nc.sync.dma_start(out=isb[:], in_=idd[:])
            vsb = const.tile([128, 128], F32)
            nc.sync.dma_start(out=vsb[:], in_=ivd[:])

            def emit_head(base):
                """DMA-in + forward transposes + copy to SBUF; returns x0sb."""
                gt = []
                for s in range(2):
                    # 4 extra overlapped columns so every transpose slice is
                    # 128 wide -> all 128 psum partitions get (finite) data
                    g = gpool.tile([128, G_FREE + 4], F32, tag="g")
                    start = (base + s * TILE_ROWS) * D
                    src = bass.AP(
                        tensor=xs,
                        offset=start,
                        ap=[[G_FREE, 128], [1, G_FREE + 4]],
                    )
                    nc.sync.dma_start(out=g[:], in_=src)
                    gt.append(g)
                x0sb = x0pool.tile([128, 2 * NFREE], F32R, tag="x0")
                for s in range(2):
                    x0 = pss.tile([128, NFREE], F32, tag="pss")
                    for t in range(4):
                        nc.tensor.transpose(
                            x0[:, 128 * t : 128 * (t + 1)],
                            gt[s][:, 125 * t : 125 * t + 128],
                            isb[:],
                        )
                    nc.scalar.copy(
                        out=x0sb[:, s * NFREE : (s + 1) * NFREE],
                        in_=x0[:],
                    )
                return x0sb

            def emit_layer(l, st):
                """One layer for one pair; st = (e1_prev, r_prev) -> new st."""
                e1_prev, r_prev = st
                wl = wsb[:, l * 128 : (l + 1) * 128]
                bl = bsb[:, l : l + 1]
                z = ps.tile([128, 2 * NFREE], F32, tag="ps")
                for h in range(2):
                    sl = slice(h * NFREE, (h + 1) * NFREE)
                    if e1_prev is None:
                        nc.tensor.matmul(
                            z[:, sl], wl, r_prev[:, sl], start=True, stop=True
                        )
                    else:
                        nc.tensor.matmul(
                            z[:, sl], wl, e1_prev[:, sl], start=True, stop=False
                        )
                        nc.tensor.matmul(
                            z[:, sl], wl, r_prev[:, sl], start=False, stop=True
                        )
                e_raw = epool.tile([128, 2 * NFREE], F32, tag="eraw")
                nc.scalar.activation(
                    out=e_raw[:],
                    in_=z[:],
                    func=mybir.ActivationFunctionType.Exp,
                    bias=bl,
                    scale=1.0,
                )
                r = rpool.tile([128, 2 * NFREE], F32R, tag="r")
                nc.vector.tensor_scalar(
                    out=r[:],
                    in0=z[:],
                    scalar1=bl,
                    scalar2=0.0,
                    op0=mybir.AluOpType.add,
                    op1=mybir.AluOpType.max,
                )
                e1 = e1pool.tile([128, 2 * NFREE], F32R, tag="e1")
                eng = nc.vector if (l % 32) < int(MIN_ON_DVE * 32) else nc.gpsimd
                eng.tensor_scalar(
                    out=e1[:],
                    in0=e_raw[:],
                    scalar1=1.0,
                    scalar2=-1.0,
                    op0=mybir.AluOpType.min,
                    op1=mybir.AluOpType.add,
                )
                return (e1, r)

            def emit_tail(base, st):
                e1_prev, r_prev = st
                hsb = hpool.tile([128, 2 * NFREE], F32, tag="h")
                for s in range(2):
                    hps = pss.tile([128, NFREE], F32, tag="pss")
                    for t in range(4):
                        osl = slice(125 * t, 125 * (t + 1))
                        csl = slice(s * NFREE + 128 * t, s * NFREE + 128 * (t + 1))
                        nc.tensor.matmul(
                            hps[:, osl],
                            e1_prev[0 : P_USED + 1, csl].bitcast(F32),
                            vsb[0 : P_USED + 1, 0:P_USED],
                            start=True,
                            stop=False,
                        )
                        nc.tensor.matmul(
                            hps[:, osl],
                            r_prev[0 : P_USED + 1, csl].bitcast(F32),
                            vsb[0 : P_USED + 1, 0:P_USED],
                            start=False,
                            stop=True,
                        )
                    nc.scalar.copy(
                        out=hsb[:, s * NFREE : s * NFREE + G_FREE],
                        in_=hps[:, 0:G_FREE],
                    )
                    start = (base + s * TILE_ROWS) * D
                    dst = ys[start : start + 128 * G_FREE].rearrange(
                        "(p f) -> p f", f=G_FREE
                    )
                    nc.sync.dma_start(
                        out=dst, in_=hsb[:, s * NFREE : s * NFREE + G_FREE]
                    )

            # interleave K_ILV pairs layer-by-layer so the engines see
            # independent work and PSUM slots rotate between pairs
            for g0 in range(0, n_pairs, K_ILV):
                group = list(range(g0, min(g0 + K_ILV, n_pairs)))
                bases = [
                    min(p * PAIR_ROWS, rows_per_core - PAIR_ROWS) for p in group
                ]
                states = [(None, emit_head(b)) for b in bases]
                for l in range(N_LAYERS):
                    for i in range(len(group)):
                        states[i] = emit_layer(l, states[i])
                for b, st in zip(bases, states):
                    emit_tail(b, st)

    _split_multiwaits(nc)
    return nc


_BASS_CACHE = {}


def _get_bass(rows_per_core):
    if rows_per_core not in _BASS_CACHE:
        _BASS_CACHE[rows_per_core] = _build_bass(rows_per_core)
    return _BASS_CACHE[rows_per_core]


# ------------------------------------------------------- entry point
def kernel(x, enc_params, dec_params, _rows_per_core=None, _collect=None):
    from concourse.bass_utils import run_bass_kernel_spmd

    x = np.asarray(x, np.float32)
    n_rows = x.shape[0]
    rows_per_core = _rows_per_core or (n_rows // N_CORES)
    wblk, bvec, ident, invsel = _host_constants(enc_params, dec_params)
    nc = _get_bass(rows_per_core)

    in_maps = []
    for c in range(N_CORES):
        shard = np.empty(rows_per_core * D + 4, np.float32)
        shard[: rows_per_core * D] = x[
            c * rows_per_core : (c + 1) * rows_per_core
        ].reshape(-1)
        shard[rows_per_core * D :] = 0.0
        in_maps.append(
            {
                "xs": shard,
                "wblk": wblk,
                "bias": bvec,
                "ident": ident,
                "invsel": invsel,
            }
        )
    res = run_bass_kernel_spmd(nc, in_maps, core_ids=list(range(N_CORES)))
    if _collect is not None:
        _collect.append(res)
    y = np.concatenate(
        [res.results[c]["ys"].reshape(rows_per_core, D) for c in range(N_CORES)],
        axis=0,
    )
    return y


# revision 13
# speedup vs baseline: 1.1151x; 1.1151x over previous
"""Trainium2 Bass kernel for nn_Autoencoder_78288663872074.

Autoencoder MLP: 32 Dense(ELU) layers, widths <= 5, over a 4194304x5 batch.

Strategy
--------
- Pure data parallel over 8 cores (524288 rows each).
- On each core, rows are packed into a block-diagonal layout: 25 independent
  5-feature blocks on 125 partitions x 512 batch columns per tile
  (12800 rows/tile).  Each Dense layer is then ONE 128x128 matmul against a
  block-diagonal weight matrix (25 copies of the 5x5 padded layer weight).
- ELU(u) = relu(u) + (min(exp(u), 1) - 1) is kept as TWO zero-centered
  tensors r and e1 whose sum is exactly the activation (both small -> safe
  to round to fp32r; never carry a +1 offset in fp32r).  Per layer:
     PE  : z = Wblk^T e1_prev + Wblk^T r_prev   (PSUM accumulate, fp32r)
     ACT : e_raw = Exp(z + B)                    (bias is free on ACT)
     DVE : r  = max(z + B, 0)                    (dual-op tensor_scalar)
     POOL: e1 = min(e_raw, 1) - 1                (gpsimd dual-op tensor_scalar)
  so the four engines each carry one pass per layer; K_ILV pairs of tiles are
  interleaved layer-by-layer so the engines always see independent work.
- Matmuls use float32r (1 PE cycle/row vs 4 for fp32; ~3e-4 per-store
  rounding -> ~1.6e-3 final norm rel err).
- Entry/exit to the block layout is done with PE transpose(-matmuls) against
  identity matrices, so all DRAM DMA is fully contiguous.
"""

import sys

sys.path.insert(0, "/opt/trn_rl_repo")

import numpy as np

import concourse.bass as bass
import concourse.mybir as mybir
import concourse.tile as tile
from concourse.vector_clock import ScopedClock

# ---------------------------------------------------------------- constants
BATCH = 4_194_304
D = 5
N_CORES = 8
ROWS_PER_CORE = BATCH // N_CORES          # 524288
NB = 25                                   # 5-feature blocks per matmul
P_USED = NB * D                           # 125 partitions carry data
NFREE = 512                               # batch columns per tile
TILE_ROWS = NB * NFREE                    # 12800 rows per tile
PAIR_ROWS = 2 * TILE_ROWS                 # 25600 rows per pair
G_FREE = 100 * D                          # 500 fp32 per partition per tile
N_LAYERS = 32

F32 = mybir.dt.float32
F32R = mybir.dt.float32r  # TF32-like: PE runs 1 cycle/row (vs 4 for fp32) at N>=256


# ------------------------------------------------------- tail-drain fix
def _patch_tile_drain():
    """walrus in this container accepts a single sync-wait per instruction;
    spread the kernel-tail drain waits over one-wait SP nops."""

    def _drain_and_barrier_split(self, tick_clock, wait_clock):
        nc = self.nc
        probe = nc.sync.nop(nofuse=True)
        wait_clock.add_sem_waits(
            probe.ins, ScopedClock({None: tick_clock.global_clock})
        )
        si = probe.ins.sync_info
        ow = list(si.on_wait or []) if si else []
        if len(ow) > 1:
            si.on_wait = ow[:1]
            for w in ow[1:]:
                n = nc.sync.nop(nofuse=True)
                nsi = n.ins.sync_info
                if nsi is None:
                    n.ins.sync_info = mybir.SyncInfo(on_wait=[w], on_update=[])
                else:
                    nsi.on_wait = [w]
        nc.sync.drain()
        nc.all_engine_barrier()
        assert self.sems is not None
        popped = nc._tile_sem_poison_stack.pop()
        assert popped is self._sem_poison
        nc.clear_and_free_semaphores(list(self.sems.allocated().values()))
        nc.all_engine_barrier()

    tile.TileContext._drain_and_barrier = _drain_and_barrier_split


_patch_tile_drain()


def _split_multiwaits(nc):
    """Hoist extra sync-waits onto same-engine NOPs (1 wait per instruction)."""
    n_split = 0
    for fn in nc.m.functions:
        for blk in fn.blocks:
            new = []
            for inst in blk.instructions:
                si = inst.sync_info
                ow = list(si.on_wait) if (si and si.on_wait) else []
                if len(ow) > 1:
                    for i, w in enumerate(ow[:-1]):
                        new.append(
                            mybir.InstNoOp(
                                name=f"{inst.name}-wsplit{i}",
                                engine=inst.engine,
                                ins=[],
                                outs=[],
                                sync_info=mybir.SyncInfo(on_wait=[w], on_update=[]),
                            )
                        )
                        n_split += 1
                    si.on_wait = [ow[-1]]
                new.append(inst)
            blk.instructions[:] = new
    return n_split


# ------------------------------------------------------- host-side weights
def _layer_list(enc_params, dec_params):
    layers = []
    for params in (enc_params, dec_params):
        for W, b in params:
            layers.append((np.asarray(W, np.float32), np.asarray(b, np.float32)))
    assert len(layers) == N_LAYERS
    return layers


def _host_constants(enc_params, dec_params):
    """Build block-diag weights, corrected per-partition biases, selectors."""
    layers = _layer_list(enc_params, dec_params)

    wblk = np.zeros((128, N_LAYERS * 128), np.float32)  # sbuf layout [p, l*128+m]
    bvec = np.zeros((128, N_LAYERS), np.float32)
    for l, (W, b) in enumerate(layers):
        d_in, d_out = W.shape
        W5 = np.zeros((D, D), np.float32)
        W5[:d_in, :d_out] = W
        b5 = np.zeros((D,), np.float32)
        b5[:d_out] = b
        B5 = b5  # activations are stored as (e1-1, r); their sum is exactly ELU
        blk = np.zeros((128, 128), np.float32)
        for bi in range(NB):
            blk[D * bi : D * bi + D, D * bi : D * bi + D] = W5
        wblk[:, l * 128 : (l + 1) * 128] = blk
        bvec[:P_USED, l] = np.tile(B5, NB)

    ident = np.zeros((128, 128), np.float32)
    np.fill_diagonal(ident[:128, :128], 1.0)

    invsel = ident.copy()
    return wblk, bvec, ident, invsel


# ------------------------------------------------------- device program
K_ILV = 3          # pairs interleaved layer-by-layer (PSUM-slot limited)
MIN_DVE_COLS = 128  # columns of the min-op handled by DVE (rest on POOL)


def _build_bass(rows_per_core):
    n_pairs = (rows_per_core + PAIR_ROWS - 1) // PAIR_ROWS
    assert rows_per_core >= PAIR_ROWS

    nc = bass.Bass(
        "TRN2",
        target_bir_lowering=False,
        debug=False,
        enable_asserts=False,
        num_devices=1,
    )
    xs = nc.dram_tensor("xs", [rows_per_core * D + 4], F32, kind="ExternalInput")
    wd = nc.dram_tensor("wblk", [128, N_LAYERS * 128], F32R, kind="ExternalInput")
    bd = nc.dram_tensor("bias", [128, N_LAYERS], F32, kind="ExternalInput")
    idd = nc.dram_tensor("ident", [128, 128], F32, kind="ExternalInput")
    ivd = nc.dram_tensor("invsel", [128, 128], F32, kind="ExternalInput")
    ys = nc.dram_tensor("ys", [rows_per_core * D], F32, kind="ExternalOutput")

    with tile.TileContext(nc) as tc:
        with (
            tc.tile_pool(name="const", bufs=1) as const,
            tc.tile_pool(name="g", bufs=4 * K_ILV) as gpool,
            tc.tile_pool(name="x0sb", bufs=K_ILV + 1) as x0pool,
            tc.tile_pool(name="eraw", bufs=2 * K_ILV + 1) as epool,
            tc.tile_pool(name="rr", bufs=2 * K_ILV + 2) as rpool,
            tc.tile_pool(name="e1", bufs=2 * K_ILV + 2) as e1pool,
            tc.tile_pool(name="hsb", bufs=K_ILV + 1) as hpool,
            tc.tile_pool(name="ps", bufs=3, space="PSUM") as ps,
            tc.tile_pool(name="pss", bufs=2, space="PSUM") as pss,
        ):
            wsb = const.tile([128, N_LAYERS * 128], F32R)
            nc.sync.dma_start(out=wsb[:], in_=wd[:])
            bsb = const.tile([128, N_LAYERS], F32)
            nc.sync.dma_start(out=bsb[:], in_=bd[:])
            isb = const.tile([128, 128], F32)
            nc.sync.dma_start(out=isb[:], in_=idd[:])
            vsb = const.tile([128, 128], F32)
            nc.sync.dma_start(out=vsb[:], in_=ivd[:])

            def emit_head(base):
                """DMA-in + forward transposes + copy to SBUF; returns x0sb."""
                gt = []
                for s in range(2):
                    # 4 extra overlapped columns so every transpose slice is
                    # 128 wide -> all 128 psum partitions get (finite) data
                    g = gpool.tile([128, G_FREE + 4], F32, tag="g")
                    start = (base + s * TILE_ROWS) * D
                    src = bass.AP(
                        tensor=xs,
                        offset=start,
                        ap=[[G_FREE, 128], [1, G_FREE + 4]],
                    )
                    nc.sync.dma_start(out=g[:], in_=src)
                    gt.append(g)
                x0sb = x0pool.tile([128, 2 * NFREE], F32R, tag="x0")
                for s in range(2):
                    x0 = pss.tile([128, NFREE], F32, tag="pss")
                    for t in range(4):
                        nc.tensor.transpose(
                            x0[:, 128 * t : 128 * (t + 1)],
                            gt[s][:, 125 * t : 125 * t + 128],
                            isb[:],
                        )
                    nc.scalar.copy(
                        out=x0sb[:, s * NFREE : (s + 1) * NFREE],
                        in_=x0[:],
                    )
                return x0sb

            def emit_layer(l, st):
                """One layer for one pair; st = (e1_prev, r_prev) -> new st."""
                e1_prev, r_prev = st
                wl = wsb[:, l * 128 : (l + 1) * 128]
                bl = bsb[:, l : l + 1]
                z = ps.tile([128, 2 * NFREE], F32, tag="ps")
                for h in range(2):
                    sl = slice(h * NFREE, (h + 1) * NFREE)
                    if e1_prev is None:
                        nc.tensor.matmul(
                            z[:, sl], wl, r_prev[:, sl], start=True, stop=True
                        )
                    else:
                        nc.tensor.matmul(
                            z[:, sl], wl, e1_prev[:, sl], start=True, stop=False
                        )
                        nc.tensor.matmul(
                            z[:, sl], wl, r_prev[:, sl], start=False, stop=True
                        )
                e_raw = epool.tile([128, 2 * NFREE], F32, tag="eraw")
                nc.scalar.activation(
                    out=e_raw[:],
                    in_=z[:],
                    func=mybir.ActivationFunctionType.Exp,
                    bias=bl,
                    scale=1.0,
                )
                r = rpool.tile([128, 2 * NFREE], F32R, tag="r")
                nc.vector.tensor_scalar(
                    out=r[:],
                    in0=z[:],
                    scalar1=bl,
                    scalar2=0.0,
                    op0=mybir.AluOpType.add,
                    op1=mybir.AluOpType.max,
                )
                e1 = e1pool.tile([128, 2 * NFREE], F32R, tag="e1")
                # split the min between DVE (2x-mode SBUF slice) and POOL so
                # neither engine is the lone bottleneck (cost-model balanced)
                C = MIN_DVE_COLS
                if C > 0:
                    nc.vector.tensor_scalar(
                        out=e1[:, 0:C], in0=e_raw[:, 0:C], scalar1=1.0,
                        scalar2=-1.0, op0=mybir.AluOpType.min,
                        op1=mybir.AluOpType.add,
                    )
                if C < 2 * NFREE:
                    nc.gpsimd.tensor_scalar(
                        out=e1[:, C:], in0=e_raw[:, C:], scalar1=1.0,
                        scalar2=-1.0, op0=mybir.AluOpType.min,
                        op1=mybir.AluOpType.add,
                    )
                return (e1, r)

            def emit_tail(base, st):
                e1_prev, r_prev = st
                hsb = hpool.tile([128, 2 * NFREE], F32, tag="h")
                for s in range(2):
                    hps = pss.tile([128, NFREE], F32, tag="pss")
                    for t in range(4):
                        osl = slice(125 * t, 125 * (t + 1))
                        csl = slice(s * NFREE + 128 * t, s * NFREE + 128 * (t + 1))
                        nc.tensor.matmul(
                            hps[:, osl],
                            e1_prev[0 : P_USED + 1, csl].bitcast(F32),
                            vsb[0 : P_USED + 1, 0:P_USED],
                            start=True,
                            stop=False,
                        )
                        nc.tensor.matmul(
                            hps[:, osl],
                            r_prev[0 : P_USED + 1, csl].bitcast(F32),
                            vsb[0 : P_USED + 1, 0:P_USED],
                            start=False,
                            stop=True,
                        )
                    nc.scalar.copy(
                        out=hsb[:, s * NFREE : s * NFREE + G_FREE],
                        in_=hps[:, 0:G_FREE],
                    )
                    start = (base + s * TILE_ROWS) * D
                    dst = ys[start : start + 128 * G_FREE].rearrange(
                        "(p f) -> p f", f=G_FREE
                    )
                    nc.sync.dma_start(
                        out=dst, in_=hsb[:, s * NFREE : s * NFREE + G_FREE]
                    )

            # interleave K_ILV pairs layer-by-layer so the engines see
            # independent work and PSUM slots rotate between pairs
            for g0 in range(0, n_pairs, K_ILV):
                group = list(range(g0, min(g0 + K_ILV, n_pairs)))
                bases = [
                    min(p * PAIR_ROWS, rows_per_core - PAIR_ROWS) for p in group
                ]
                states = [(None, emit_head(b)) for b in bases]
                for l in range(N_LAYERS):
                    for i in range(len(group)):
                        states[i] = emit_layer(l, states[i])
                for b, st in zip(bases, states):
                    emit_tail(b, st)

    _split_multiwaits(nc)
    return nc


_BASS_CACHE = {}


def _get_bass(rows_per_core):
    if rows_per_core not in _BASS_CACHE:
        _BASS_CACHE[rows_per_core] = _build_bass(rows_per_core)
    return _BASS_CACHE[rows_per_core]


# ------------------------------------------------------- entry point
def kernel(x, enc_params, dec_params, _rows_per_core=None, _collect=None):
    from concourse.bass_utils import run_bass_kernel_spmd

    x = np.asarray(x, np.float32)
    n_rows = x.shape[0]
    rows_per_core = _rows_per_core or (n_rows // N_CORES)
    wblk, bvec, ident, invsel = _host_constants(enc_params, dec_params)
    nc = _get_bass(rows_per_core)

    in_maps = []
    for c in range(N_CORES):
        shard = np.empty(rows_per_core * D + 4, np.float32)
        shard[: rows_per_core * D] = x[
            c * rows_per_core : (c + 1) * rows_per_core
        ].reshape(-1)
        shard[rows_per_core * D :] = 0.0
        in_maps.append(
            {
                "xs": shard,
                "wblk": wblk,
                "bias": bvec,
                "ident": ident,
                "invsel": invsel,
            }
        )
    res = run_bass_kernel_spmd(nc, in_maps, core_ids=list(range(N_CORES)))
    if _collect is not None:
        _collect.append(res)
    y = np.concatenate(
        [res.results[c]["ys"].reshape(rows_per_core, D) for c in range(N_CORES)],
        axis=0,
    )
    return y
